# revision 30
# baseline (speedup 1.0000x reference)
"""MoE-LoRA layer (nn_MoELoRALayer) as a Bass/Tile kernel for 8 Trainium2 cores.

Computation (per token n):
    logits = x @ W_router.T                    # [N, 8]
    combine = renorm(top2(softmax(logits)))    # [N, 8]
    h       = x @ A_cat.T                      # [N, 128]   (8 experts x rank 16)
    hw      = h * combine_expanded             # [N, 128]
    out     = x @ W_base.T + b + 2.0 * hw @ B_cat.T

Sharding: data-parallel over tokens (1024 per core), weights replicated.

v2 structure (vs the bf16 baseline):
  * Mixed-precision base GEMM: k-tiles 0-23 run bf16, k-tiles 24-31 run as 4
    fp8(e4m3) DoubleRow matmuls (2 k-tiles per instruction, 2x PE throughput).
    All W-side operands are pre-scaled x256 host-side so the fp8 W values sit
    in e4m3's normal range; the PSUM drain multiplies by 1/256 fused into the
    bias add. Measured end-to-end rel err ~1.6e-2 (gate 2e-2).
  * Accumulators open with the base GEMM and close with the LoRA up-proj
    (stop=True), so no o-tile ever waits on the routing chain at its start.
  * Phase 1 (router logits + LoRA down-proj, both 512-token blocks) is
    interleaved k-by-k with o-tile 0's base K-loop for token tiles 0-3, so
    the PE consumes the incoming x stream at DMA rate with no startup stall.
    PSUM: 4 banks phase 1 + 4 banks accs. After the k-loop, token tiles 4,5
    run from a persistent W cache while the routing math (DVE/ACT) completes
    under them; token tiles 6,7 of o-tile 0 are deferred to the very end.
  * Routing math per 128-token chunk: 8 DVE ops + 2 ACT ops (sigmoid trick
    folds exp/1+exp/reciprocal into one activation).

Host-side layout prep (part of sharding):
    xt   [32, 128, 1024]  = x_shard.T bf16, K-tile major
    x8   [4, 128, 2, 1024] = e4m3 x_shard.T for k 3072..4095, DoubleRow pairs
    wtb  [24, 128, 4096]  = (W_base.T * 256)[0:3072] bf16, K-tile major
    w8   [4, 128, 2, 4096] = e4m3 (W_base.T * 256)[3072:4096], DoubleRow pairs
    at   [128, 32, 128]   = A.transpose(2,0,1) packed per K-tile (j = e*16+r)
    bft  [128, 4096]      = 2.0 * 256 * B.transpose(0,2,1).reshape(128, 4096)
    pkb  [128, 384]       = packed W_router.T (per K-tile) + expansion matrix
    ident [128, 128]      = identity for PE transposes
"""

import numpy as np

import concourse.bacc as bacc
import concourse.bass as bass
import concourse.mybir as mybir
import concourse.tile as tile
from concourse.bass_utils import run_bass_kernel_spmd

N_CORES = 8
D_IN = 4096
D_OUT = 4096
N_EXP = 8
R = 16
J = N_EXP * R           # 128
SCALING = 2.0
S = 256.0               # PSUM pre-scale folded into all W-side operands
TOK = 1024              # tokens per core
K_TILES = D_IN // 128   # 32
KB = 22                 # bf16 k-tiles (0..21)
KF = 5                  # fp8 DoubleRow super-tiles (k-tiles 22..31, 2 per)
N_TILES = TOK // 128    # 8
O_TILES = D_OUT // 512  # 8
BLK = 512

F32 = mybir.dt.float32
BF16 = mybir.dt.bfloat16
F8 = mybir.dt.float8e4

_CACHE = {}


def _build_program(finalize=True):
    key = ("nc", finalize)
    if key in _CACHE:
        return _CACHE[key]

    nc = bacc.Bacc(trn_type="TRN2")

    xt_d = nc.dram_tensor("xt", [K_TILES, 128, TOK], BF16, kind="ExternalInput")
    x8_d = nc.dram_tensor("x8", [KF, 128, 2, TOK], F8, kind="ExternalInput")
    wtb_d = nc.dram_tensor("wtb", [KB, 128, D_OUT], BF16, kind="ExternalInput")
    w8_d = nc.dram_tensor("w8", [KF, 128, 2, D_OUT], F8, kind="ExternalInput")
    at_d = nc.dram_tensor("at", [128, KB, J], BF16, kind="ExternalInput")
    at8_d = nc.dram_tensor("at8", [KF, 128, 2, J], F8, kind="ExternalInput")
    bft_d = nc.dram_tensor("bft", [J, D_OUT], BF16, kind="ExternalInput")
    bvec_d = nc.dram_tensor("bvec", [D_OUT], F32, kind="ExternalInput")
    pkb_d = nc.dram_tensor("pkb", [128, 384], BF16, kind="ExternalInput")
    id_d = nc.dram_tensor("ident", [128, 128], F32, kind="ExternalInput")
    out_d = nc.dram_tensor("out", [TOK, D_OUT], F32, kind="ExternalOutput")

    out_ap = out_d[:]
    mm = nc.tensor.matmul
    DR = mybir.MatmulPerfMode.DoubleRow

    with tile.TileContext(nc) as tc:
        with (
            tc.tile_pool(name="xt_pool", bufs=8) as xt_pool,
            tc.tile_pool(name="res", bufs=1) as res,
            tc.tile_pool(name="wt_pool", bufs=2) as wt_pool,
            tc.tile_pool(name="w8_pool", bufs=6) as w8_pool,
            tc.tile_pool(name="out_pool", bufs=6) as out_pool,
            tc.tile_pool(name="rsm", bufs=2) as rsm,
            tc.tile_pool(name="rbig", bufs=2) as rbig,
            tc.tile_pool(name="ps", bufs=8, space="PSUM") as ps,
        ):
            # ---- resident loads: phase-1 operands first so the k-loop can
            # start as soon as the first xt chunk lands ----
            pkr = res.tile([128, 384], BF16)
            nc.sync.dma_start(out=pkr, in_=pkb_d[:])
            pkf = res.tile([128, 128], F32)
            nc.sync.dma_start(out=pkf, in_=id_d[:])
            at_sb = res.tile([128, KB, J], BF16)
            nc.sync.dma_start(out=at_sb[:, 0:16, :], in_=at_d[:, 0:16, :])
            at8_sb = res.tile([128, KF, 2, J], F8)
            for t in range(KF):
                nc.sync.dma_start(out=at8_sb[:, t, :, :], in_=at8_d[t])
            wrt_sb = pkr[:, 0:256].rearrange("p (k e) -> p k e", e=N_EXP)
            emat_sb = pkr[0:N_EXP, 256:384]
            ident_sb = pkf
            hwt_sb = res.tile([J, TOK], BF16)
            w80 = res.tile([128, KF, 2, 512], F8)
            x8_sb = res.tile([128, KF, 2, TOK], F8)

            # x stream: chunk 0 split per K-tile so the PE can start on k=0
            # without waiting a full 1MB chunk; later chunks 4 K-tiles wide,
            # alternating queues. o0's W cache DMAs interleave with the x
            # chunks on the scalar queue so neither stream starves the other.
            wb0 = res.tile([128, KB, 512], BF16)
            xts = []
            xt_chunks = []
            g0 = xt_pool.tile([128, 4, TOK], BF16, tag="xt", name="xt_0")
            for kk in range(4):
                nc.scalar.dma_start(
                    out=g0[:, kk, :], in_=xt_d[kk]
                )
                xts.append(g0[:, kk, :])
            xt_chunks.append(g0)
            for g in range(1, 8):
                t = xt_pool.tile([128, 4, TOK], BF16, tag="xt", name=f"xt_{g}")
                eng = nc.scalar if g % 2 == 0 else nc.sync
                eng.dma_start(out=t, in_=xt_d[g * 4:(g + 1) * 4].transpose([1, 0, 2]))
                if g % 2 == 0:
                    j = 6 * (g // 2 - 1)
                    nc.scalar.dma_start(
                        out=wb0[:, j:j + 6, :],
                        in_=wtb_d[j:j + 6, :, 0:512].transpose([1, 0, 2]),
                    )
                if g == 3:
                    nc.sync.dma_start(
                        out=at_sb[:, 16:KB, :], in_=at_d[:, 16:KB, :]
                    )
                for kk in range(4):
                    xts.append(t[:, kk, :])
                xt_chunks.append(t)
            nc.scalar.dma_start(
                out=wb0[:, 18:KB, :],
                in_=wtb_d[18:KB, :, 0:512].transpose([1, 0, 2]),
            )
            for t2 in range(KF):
                nc.sync.dma_start(out=x8_sb[:, t2, :, :], in_=x8_d[t2])
                nc.sync.dma_start(out=w80[:, t2, :, :], in_=w8_d[t2, :, :, 0:512])
            bft_sb = res.tile([J, D_OUT], BF16)
            nc.scalar.dma_start(out=bft_sb, in_=bft_d[:])
            bias_sb = res.tile([128, D_OUT], F32)
            nc.gpsimd.dma_start(
                out=bias_sb, in_=bvec_d[:].partition_broadcast(128)
            )
            # bias arrives on a SWDGE queue; observe it on the DVE clock once.
            btch = rsm.tile([1, 1], F32, tag="btch")
            nc.vector.tensor_copy(out=btch, in_=bias_sb[0:1, 0:1])

            # ---- PSUM slot plan (single tag, bufs=8; allocation order IS the
            # slot order):
            #   slots 0-3: pr0, ph0, pr1, ph1   (phase 1)
            #   slots 4-7: o0 accs n0..n3
            # then acc4->slot0, acc5->slot1, scr0->slot2, scr1->slot3, and all
            # later accs keep cycling 4,5,6,7,0,1,2,3.
            prsh = ps.tile([128, 512], F32, tag="ps", name="prsh")
            ph0 = ps.tile([128, 512], F32, tag="ps", name="ph0")
            ph1 = ps.tile([128, 512], F32, tag="ps", name="ph1")
            acc04 = [
                ps.tile([128, 512], F32, tag="ps", name=f"acc0_{n}")
                for n in range(5)
            ]
            # both blocks' router logits share one bank at partition offsets
            # 0 and 32 (legal matmul tile positions). PSUM start=True zeroing
            # is bank-granular, so the bank is zeroed once by the DVE and
            # every router matmul accumulates with start=False.
            pr0 = prsh[0:N_EXP, :]
            pr1 = prsh[32:32 + N_EXP, :]
            nc.vector.memset(prsh, 0.0)

            def base_mms(acc, n, wtiles, w8tiles, close=False):
                """Full mixed-precision base K-loop for token tile n."""
                nsl = slice(n * 128, (n + 1) * 128)
                for k in range(KB):
                    mm(acc, xts[k][:, nsl], wtiles[k], start=(k == 0), stop=False)
                for t in range(KF):
                    mm(acc, x8_sb[:, t, :, nsl], w8tiles[t],
                       start=False, stop=False, perf_mode=DR)

            wb0_tiles = [wb0[:, k, :] for k in range(KB)]
            w80_tiles = [w80[:, t, :, :] for t in range(KF)]

            # ---- phase A: fused phase-1 + o0(n0..3) K-loop. o0's fp8
            # DoubleRow mms run in the first k-slots (their small operand
            # tensors arrive before the bf16 x stream ramps); o0's bf16 mms
            # trail phase-1 by 8 k-tiles in the FIFO so a late W-cache chunk
            # can never head-of-line-block the router/A stream. ----
            LAG = 8
            for ks in range(K_TILES + LAG):
                if ks < K_TILES:
                    k = ks
                    sp = (k == K_TILES - 1)
                    mm(pr0, wrt_sb[:, k, :], xts[k][:, 0:512],
                       start=False, stop=sp, skip_group_check=True)
                    if k < KB:
                        mm(ph0, at_sb[:, k, :], xts[k][:, 0:512],
                           start=(k == 0), stop=False)
                    elif (k - KB) % 2 == 0:
                        t = (k - KB) // 2
                        mm(ph0, at8_sb[:, t, :, :], x8_sb[:, t, :, 0:512],
                           start=False, stop=(t == KF - 1), perf_mode=DR)
                    mm(pr1, wrt_sb[:, k, :], xts[k][:, 512:1024],
                       start=False, stop=sp, skip_group_check=True)
                    if k < KB:
                        mm(ph1, at_sb[:, k, :], xts[k][:, 512:1024],
                           start=(k == 0), stop=False)
                    elif (k - KB) % 2 == 0:
                        t = (k - KB) // 2
                        mm(ph1, at8_sb[:, t, :, :], x8_sb[:, t, :, 512:1024],
                           start=False, stop=(t == KF - 1), perf_mode=DR)
                if ks >= LAG:
                    kb = ks - LAG
                    if kb < KB:
                        for n in range(5):
                            mm(acc04[n], xts[kb][:, n * 128:(n + 1) * 128],
                               wb0_tiles[kb], start=(kb == 0), stop=False)
                    elif (kb - KB) % 2 == 0:
                        t = (kb - KB) // 2
                        for n in range(5):
                            mm(acc04[n], x8_sb[:, t, :, n * 128:(n + 1) * 128],
                               w80_tiles[t], start=False, stop=False,
                               perf_mode=DR)

            # ---- phase B allocations ----
            acc5 = ps.tile([128, 512], F32, tag="ps", name="acc5")
            acc6 = ps.tile([128, 512], F32, tag="ps", name="acc6")
            scr0 = ps.tile([128, 512], F32, tag="ps", name="scr0")

            # drain phase-1 PSUM to SBUF (frees slots 0-2)
            lg0 = rbig.tile([N_EXP, BLK], F32, tag="lg", name="lg0")
            nc.vector.tensor_copy(out=lg0, in_=pr0)
            lg1 = rbig.tile([N_EXP, BLK], F32, tag="lg", name="lg1")
            nc.vector.tensor_copy(out=lg1, in_=pr1)
            # A operands are pre-scaled x256 (e4m3 range); undo it here.
            hs0 = rbig.tile([128, BLK], F32, tag="hs", name="hs0")
            nc.vector.tensor_scalar_mul(hs0, ph0, 1.0 / S)
            hs1 = rbig.tile([128, BLK], F32, tag="hs", name="hs1")
            nc.vector.tensor_scalar_mul(hs1, ph1, 1.0 / S)

            # forward transposes: logits chunks to token-major [128, 8]
            ltoks = []
            for i in range(8):
                b, c = i // 4, i % 4
                lg = lg0 if b == 0 else lg1
                scr = scr0
                nc.tensor.transpose(
                    out=scr[:, :N_EXP],
                    in_=lg[:, c * 128:(c + 1) * 128],
                    identity=ident_sb[:N_EXP, :N_EXP],
                )
                lt = rsm.tile([128, N_EXP], F32, tag="lt", name=f"lt_{i}", bufs=8)
                nc.vector.tensor_copy(out=lt, in_=scr[:, :N_EXP])
                ltoks.append(lt)

            # top-2 renormalized softmax weights, exact algebra:
            #   m1 = max_e l; t = l - m1; m2 = max_e (t | top1 -> -inf)
            #   combine_e = [t >= m2] * exp(t) * sigmoid(-m2)
            combs = []
            for i, lt in enumerate(ltoks):
                m1 = rsm.tile([128, 1], F32, tag="m1")
                nc.vector.tensor_reduce(
                    m1, lt, axis=mybir.AxisListType.X, op=mybir.AluOpType.max
                )
                t = rsm.tile([128, N_EXP], F32, tag="t")
                nc.vector.tensor_scalar(
                    out=t, in0=lt, scalar1=m1, scalar2=None,
                    op0=mybir.AluOpType.subtract,
                )
                eq = rsm.tile([128, N_EXP], F32, tag="eq")
                nc.vector.tensor_scalar(
                    out=eq, in0=t, scalar1=0.0, scalar2=None,
                    op0=mybir.AluOpType.is_ge,
                )
                msk = rsm.tile([128, N_EXP], F32, tag="msk")
                nc.vector.scalar_tensor_tensor(
                    out=msk, in0=eq, scalar=-1e30, in1=t,
                    op0=mybir.AluOpType.mult, op1=mybir.AluOpType.add,
                )
                m2 = rsm.tile([128, 1], F32, tag="m2")
                nc.vector.tensor_reduce(
                    m2, msk, axis=mybir.AxisListType.X, op=mybir.AluOpType.max
                )
                rec = rsm.tile([128, 1], F32, tag="rec")
                nc.scalar.activation(
                    rec, m2, mybir.ActivationFunctionType.Sigmoid, scale=-1.0
                )
                et = rsm.tile([128, N_EXP], F32, tag="et")
                nc.scalar.activation(et, t, mybir.ActivationFunctionType.Exp)
                ge = rsm.tile([128, N_EXP], F32, tag="ge")
                nc.vector.tensor_scalar(
                    out=ge, in0=t, scalar1=m2, scalar2=None,
                    op0=mybir.AluOpType.is_ge,
                )
                w = rsm.tile([128, N_EXP], F32, tag="w")
                nc.vector.tensor_tensor(
                    out=w, in0=et, in1=ge, op=mybir.AluOpType.mult
                )
                comb = rsm.tile([128, N_EXP], F32, tag="comb",
                                name=f"comb_{i}", bufs=8)
                nc.vector.tensor_scalar_mul(comb, w, rec)
                combs.append(comb)

            # token tile 5 of o0 runs while the DVE chews the routing chains
            base_mms(acc5, 5, wb0_tiles, w80_tiles)

            # combine block: transpose back + expand across ranks + fold
            def combine_block(b, hs):
                combt = rbig.tile([N_EXP, BLK], BF16, tag="ct", name=f"ct_{b}")
                for c in range(4):
                    nc.tensor.transpose(
                        out=scr0[:N_EXP, :128], in_=combs[b * 4 + c],
                        identity=ident_sb,
                    )
                    nc.vector.tensor_copy(
                        out=combt[:, c * 128:(c + 1) * 128], in_=scr0[:N_EXP, :128]
                    )
                # combine_expT[j, n] = combT[j//16, n] via emat.T @ combT
                mm(scr0, emat_sb, combt, start=True, stop=True)
                nc.vector.tensor_tensor(
                    out=hwt_sb[:, b * BLK:(b + 1) * BLK], in0=hs, in1=scr0,
                    op=mybir.AluOpType.mult,
                )

            combine_block(0, hs0)
            base_mms(acc6, 6, wb0_tiles, w80_tiles)
            combine_block(1, hs1)

            def drain(acc, n, osl, name, war_close=True):
                osb = out_pool.tile([128, 512], F32, tag="ob", name=name)
                nc.vector.scalar_tensor_tensor(
                    out=osb, in0=acc, scalar=1.0 / S, in1=bias_sb[:, osl],
                    op0=mybir.AluOpType.mult, op1=mybir.AluOpType.add,
                )
                nc.sync.dma_start(
                    out=out_ap[n * 128:(n + 1) * 128, osl], in_=osb
                )
                if war_close:
                    # WAR closer: makes the DVE (not the outbound DMA queue)
                    # the releaser of this staging slot.
                    nc.vector.memset(osb[0:1, 0:1], 0.0)

            # close + drain o0 n0..6
            o0sl = slice(0, 512)
            accs_o0 = acc04 + [acc5, acc6]
            for n in range(7):
                mm(accs_o0[n], hwt_sb[:, n * 128:(n + 1) * 128],
                   bft_sb[:, o0sl], start=False, stop=True)
            for n in range(7):
                drain(accs_o0[n], n, o0sl, f"ob_0_{n}")

            # ---- o-tiles 1..7: streamed W, base-first accumulators. W comes
            # in 8-k-tile chunk DMAs (3 triggers per o-tile) so trigger
            # execution never paces the K-loop; bufs=2 bounds how far the
            # stream can run ahead of consumption (no startup HBM theft). ----
            for o in range(1, O_TILES):
                osl = slice(o * 512, (o + 1) * 512)
                w8s = []
                for t in range(KF):
                    w8t = w8_pool.tile([128, 2, 512], F8, tag="w8",
                                       name=f"w8_{o}_{t}")
                    nc.sync.dma_start(out=w8t, in_=w8_d[t, :, :, osl])
                    w8s.append(w8t)
                accs = [
                    ps.tile([128, 512], F32, tag="ps", name=f"acc_{o}_{n}")
                    for n in range(N_TILES)
                ]
                for kc, k0 in enumerate(range(0, KB, 8)):
                    kw = min(8, KB - k0)
                    wtc = wt_pool.tile([128, 8, 512], BF16, tag="wt",
                                       name=f"wt_{o}_{kc}",
                                       padded_shape=[128, 8, 512])
                    nc.scalar.dma_start(
                        out=wtc[:, 0:kw, :],
                        in_=wtb_d[k0:k0 + kw, :, osl].transpose([1, 0, 2]),
                    )
                    for n in range(N_TILES):
                        for kk in range(kw):
                            k = k0 + kk
                            mm(accs[n], xts[k][:, n * 128:(n + 1) * 128],
                               wtc[:, kk, :], start=(k == 0), stop=False)
                for n in range(N_TILES):
                    for t in range(KF):
                        mm(accs[n], x8_sb[:, t, :, n * 128:(n + 1) * 128],
                           w8s[t], start=False, stop=False, perf_mode=DR)
                # close+drain interleaved: DVE starts draining acc n while
                # the PE closes n+1, so PSUM slots release in a cascade.
                for n in range(N_TILES):
                    mm(accs[n], hwt_sb[:, n * 128:(n + 1) * 128],
                       bft_sb[:, osl], start=False, stop=True)
                    drain(accs[n], n, osl, f"ob_{o}_{n}")

            # ---- deferred o0 token tile 7 (from the persistent cache) ----
            accd = ps.tile([128, 512], F32, tag="ps", name="accd_7")
            base_mms(accd, 7, wb0_tiles, w80_tiles)
            mm(accd, hwt_sb[:, 7 * 128:8 * 128],
               bft_sb[:, o0sl], start=False, stop=True)
            drain(accd, 7, o0sl, "ob_d_7", war_close=False)

    if finalize:
        nc.finalize()
    _CACHE[key] = nc
    return nc


def _prep_inputs(x, W_base, b_base, W_router, A, B):
    """Shard + lay out inputs for the 8 cores. Returns list of in_maps."""
    import ml_dtypes
    bf16 = ml_dtypes.bfloat16
    f8 = ml_dtypes.float8_e4m3
    x = np.asarray(x)
    W_base = np.asarray(W_base)
    b_base = np.asarray(b_base)
    W_router = np.asarray(W_router)
    A = np.asarray(A)
    B = np.asarray(B)
    x_flat = np.ascontiguousarray(x, dtype=np.float32).reshape(-1, D_IN)

    wT = W_base.T.astype(np.float32, copy=False) * S       # [4096, 4096]
    wtb = np.ascontiguousarray(
        wT[:KB * 128].reshape(KB, 128, D_OUT).astype(bf16)
    )
    w8 = np.ascontiguousarray(
        np.clip(wT[KB * 128:], -240.0, 240.0)
        .reshape(KF, 2, 128, D_OUT)
        .transpose(0, 2, 1, 3)
        .astype(f8)
    )
    atm = (
        A.astype(np.float32, copy=False)
        .transpose(2, 0, 1)
        .reshape(D_IN, J)
    ) * S
    at = np.ascontiguousarray(
        atm[:KB * 128].reshape(KB, 128, J).transpose(1, 0, 2).astype(bf16)
    )
    at8 = np.ascontiguousarray(
        np.clip(atm[KB * 128:], -240.0, 240.0)
        .reshape(KF, 2, 128, J)
        .transpose(0, 2, 1, 3)
        .astype(f8)
    )
    wrt = (
        W_router.T.astype(np.float32, copy=False)
        .reshape(K_TILES, 128, N_EXP)
        .transpose(1, 0, 2)
    )
    bft = np.ascontiguousarray(
        (SCALING * S * B.astype(np.float32, copy=False).transpose(0, 2, 1)
         .reshape(J, D_OUT)).astype(bf16)
    )
    bvec = np.ascontiguousarray(b_base, dtype=np.float32)
    # packed bf16 residents: [:, :256] wrt, [:8, 256:384] emat
    pkb = np.zeros((128, 384), dtype=bf16)
    pkb[:, 0:256] = wrt.reshape(128, K_TILES * N_EXP).astype(bf16)
    pkb[0:N_EXP, 256:384] = np.repeat(
        np.eye(N_EXP, dtype=np.float32), R, axis=1
    ).astype(bf16)
    ident = np.eye(128, dtype=np.float32)

    in_maps = []
    for c in range(N_CORES):
        shard = x_flat[c * TOK:(c + 1) * TOK]              # [1024, 4096]
        shT = np.ascontiguousarray(shard.T)                # [4096, 1024]
        xt = shT.astype(bf16).reshape(K_TILES, 128, TOK)
        x8 = np.ascontiguousarray(
            np.clip(shT[KB * 128:], -240.0, 240.0)
            .reshape(KF, 2, 128, TOK)
            .transpose(0, 2, 1, 3)
            .astype(f8)
        )
        in_maps.append({
            "xt": xt, "x8": x8, "wtb": wtb, "w8": w8, "at": at, "at8": at8,
            "bft": bft, "bvec": bvec, "pkb": pkb, "ident": ident,
        })
    return in_maps


def _run(in_maps, trace=False, **kw):
    nc = _build_program()
    return run_bass_kernel_spmd(
        nc, in_maps, core_ids=list(range(N_CORES)), trace=trace, **kw
    )


def kernel(x, W_base, b_base, W_router, A, B):
    orig_shape = np.asarray(x).shape
    in_maps = _prep_inputs(x, W_base, b_base, W_router, A, B)
    res = _run(in_maps)
    shards = [res.results[c]["out"] for c in range(N_CORES)]
    out = np.concatenate(shards, axis=0)
    return out.reshape(*orig_shape[:-1], D_OUT).astype(np.float32, copy=False)


# revision 31
# speedup vs baseline: 1.1776x; 1.1776x over previous
"""MoE-LoRA layer (nn_MoELoRALayer) as a Bass/Tile kernel for 8 Trainium2 cores.

Computation (per token n):
    logits = x @ W_router.T                    # [N, 8]
    combine = renorm(top2(softmax(logits)))    # [N, 8]
    h       = x @ A_cat.T                      # [N, 128]   (8 experts x rank 16)
    hw      = h * combine_expanded             # [N, 128]
    out     = x @ W_base.T + b + 2.0 * hw @ B_cat.T

Sharding: data-parallel over tokens (1024 per core), weights replicated.

v2 structure (vs the bf16 baseline):
  * Mixed-precision base GEMM: k-tiles 0-23 run bf16, k-tiles 24-31 run as 4
    fp8(e4m3) DoubleRow matmuls (2 k-tiles per instruction, 2x PE throughput).
    All W-side operands are pre-scaled x256 host-side so the fp8 W values sit
    in e4m3's normal range; the PSUM drain multiplies by 1/256 fused into the
    bias add. Measured end-to-end rel err ~1.6e-2 (gate 2e-2).
  * Accumulators open with the base GEMM and close with the LoRA up-proj
    (stop=True), so no o-tile ever waits on the routing chain at its start.
  * Phase 1 (router logits + LoRA down-proj, both 512-token blocks) is
    interleaved k-by-k with o-tile 0's base K-loop for token tiles 0-3, so
    the PE consumes the incoming x stream at DMA rate with no startup stall.
    PSUM: 4 banks phase 1 + 4 banks accs. After the k-loop, token tiles 4,5
    run from a persistent W cache while the routing math (DVE/ACT) completes
    under them; token tiles 6,7 of o-tile 0 are deferred to the very end.
  * Routing math per 128-token chunk: 8 DVE ops + 2 ACT ops (sigmoid trick
    folds exp/1+exp/reciprocal into one activation).

Host-side layout prep (part of sharding):
    xt   [32, 128, 1024]  = x_shard.T bf16, K-tile major
    x8   [4, 128, 2, 1024] = e4m3 x_shard.T for k 3072..4095, DoubleRow pairs
    wtb  [24, 128, 4096]  = (W_base.T * 256)[0:3072] bf16, K-tile major
    w8   [4, 128, 2, 4096] = e4m3 (W_base.T * 256)[3072:4096], DoubleRow pairs
    at   [128, 32, 128]   = A.transpose(2,0,1) packed per K-tile (j = e*16+r)
    bft  [128, 4096]      = 2.0 * 256 * B.transpose(0,2,1).reshape(128, 4096)
    pkb  [128, 384]       = packed W_router.T (per K-tile) + expansion matrix
    ident [128, 128]      = identity for PE transposes
"""

import numpy as np

import concourse.bacc as bacc
import concourse.bass as bass
import concourse.mybir as mybir
import concourse.tile as tile
from concourse.bass_utils import run_bass_kernel_spmd

N_CORES = 8
D_IN = 4096
D_OUT = 4096
N_EXP = 8
R = 16
J = N_EXP * R           # 128
SCALING = 2.0
S = 256.0               # PSUM pre-scale folded into all W-side operands
TOK = 1024              # tokens per core
K_TILES = D_IN // 128   # 32
KB = 22                 # bf16 k-tiles (0..21)
KF = 5                  # fp8 DoubleRow super-tiles (k-tiles 22..31, 2 per)
N_TILES = TOK // 128    # 8
O_TILES = D_OUT // 512  # 8
BLK = 512

F32 = mybir.dt.float32
BF16 = mybir.dt.bfloat16
F8 = mybir.dt.float8e4

_CACHE = {}


def _build_program(finalize=True):
    key = ("nc", finalize)
    if key in _CACHE:
        return _CACHE[key]

    nc = bacc.Bacc(trn_type="TRN2")

    xt_d = nc.dram_tensor("xt", [K_TILES, 128, TOK], BF16, kind="ExternalInput")
    x8_d = nc.dram_tensor("x8", [KF, 128, 2, TOK], F8, kind="ExternalInput")
    wtb_d = nc.dram_tensor("wtb", [KB, 128, D_OUT], BF16, kind="ExternalInput")
    w8_d = nc.dram_tensor("w8", [KF, 128, 2, D_OUT], F8, kind="ExternalInput")
    at_d = nc.dram_tensor("at", [128, KB, J], BF16, kind="ExternalInput")
    at8_d = nc.dram_tensor("at8", [KF, 128, 2, J], F8, kind="ExternalInput")
    bft_d = nc.dram_tensor("bft", [J, D_OUT], BF16, kind="ExternalInput")
    bvec_d = nc.dram_tensor("bvec", [D_OUT], F32, kind="ExternalInput")
    pkb_d = nc.dram_tensor("pkb", [128, 384], BF16, kind="ExternalInput")
    id_d = nc.dram_tensor("ident", [128, 128], F32, kind="ExternalInput")
    out_d = nc.dram_tensor("out", [TOK, D_OUT], F32, kind="ExternalOutput")

    out_ap = out_d[:]
    mm = nc.tensor.matmul
    DR = mybir.MatmulPerfMode.DoubleRow

    with tile.TileContext(nc) as tc:
        with (
            tc.tile_pool(name="xt_pool", bufs=8) as xt_pool,
            tc.tile_pool(name="res", bufs=1) as res,
            tc.tile_pool(name="wt_pool", bufs=2) as wt_pool,
            tc.tile_pool(name="w8_pool", bufs=6) as w8_pool,
            tc.tile_pool(name="out_pool", bufs=6) as out_pool,
            tc.tile_pool(name="rsm", bufs=2) as rsm,
            tc.tile_pool(name="rbig", bufs=2) as rbig,
            tc.tile_pool(name="ps", bufs=8, space="PSUM") as ps,
        ):
            # ---- resident loads: phase-1 operands first so the k-loop can
            # start as soon as the first xt chunk lands ----
            pkr = res.tile([128, 384], BF16)
            nc.sync.dma_start(out=pkr, in_=pkb_d[:])
            pkf = res.tile([128, 128], F32)
            nc.sync.dma_start(out=pkf, in_=id_d[:])
            at_sb = res.tile([128, KB, J], BF16)
            nc.sync.dma_start(out=at_sb[:, 0:16, :], in_=at_d[:, 0:16, :])
            at8_sb = res.tile([128, KF, 2, J], F8)
            for t in range(KF):
                nc.sync.dma_start(out=at8_sb[:, t, :, :], in_=at8_d[t])
            wrt_sb = pkr[:, 0:256].rearrange("p (k e) -> p k e", e=N_EXP)
            emat_sb = pkr[0:N_EXP, 256:384]
            ident_sb = pkf
            hwt_sb = res.tile([J, TOK], BF16)
            w80 = res.tile([128, KF, 2, 512], F8)
            x8_sb = res.tile([128, KF, 2, TOK], F8)

            # x stream: chunk 0 split per K-tile so the PE can start on k=0
            # without waiting a full 1MB chunk; later chunks 4 K-tiles wide,
            # alternating queues. o0's W cache DMAs interleave with the x
            # chunks on the scalar queue so neither stream starves the other.
            wb0 = res.tile([128, KB, 512], BF16)
            xts = []
            xt_chunks = []
            g0 = xt_pool.tile([128, 4, TOK], BF16, tag="xt", name="xt_0")
            for kk in range(4):
                nc.scalar.dma_start(
                    out=g0[:, kk, :], in_=xt_d[kk]
                )
                xts.append(g0[:, kk, :])
            xt_chunks.append(g0)
            for g in range(1, 8):
                t = xt_pool.tile([128, 4, TOK], BF16, tag="xt", name=f"xt_{g}")
                eng = nc.scalar if g % 2 == 0 else nc.sync
                eng.dma_start(out=t, in_=xt_d[g * 4:(g + 1) * 4].transpose([1, 0, 2]))
                if g % 2 == 0:
                    j = 6 * (g // 2 - 1)
                    nc.scalar.dma_start(
                        out=wb0[:, j:j + 6, :],
                        in_=wtb_d[j:j + 6, :, 0:512].transpose([1, 0, 2]),
                    )
                if g == 3:
                    nc.sync.dma_start(
                        out=at_sb[:, 16:KB, :], in_=at_d[:, 16:KB, :]
                    )
                for kk in range(4):
                    xts.append(t[:, kk, :])
                xt_chunks.append(t)
            nc.scalar.dma_start(
                out=wb0[:, 18:KB, :],
                in_=wtb_d[18:KB, :, 0:512].transpose([1, 0, 2]),
            )
            for t2 in range(KF):
                nc.sync.dma_start(out=x8_sb[:, t2, :, :], in_=x8_d[t2])
                nc.sync.dma_start(out=w80[:, t2, :, :], in_=w8_d[t2, :, :, 0:512])
            bft_sb = res.tile([J, D_OUT], BF16)
            nc.scalar.dma_start(out=bft_sb, in_=bft_d[:])
            bias_sb = res.tile([128, D_OUT], F32)
            nc.sync.dma_start(
                out=bias_sb, in_=bvec_d[:].partition_broadcast(128)
            )
            # bias arrives on a SWDGE queue; observe it on the DVE clock once.
            btch = rsm.tile([1, 1], F32, tag="btch")
            nc.vector.tensor_copy(out=btch, in_=bias_sb[0:1, 0:1])

            # ---- PSUM slot plan (single tag, bufs=8; allocation order IS the
            # slot order):
            #   slots 0-3: pr0, ph0, pr1, ph1   (phase 1)
            #   slots 4-7: o0 accs n0..n3
            # then acc4->slot0, acc5->slot1, scr0->slot2, scr1->slot3, and all
            # later accs keep cycling 4,5,6,7,0,1,2,3.
            prsh = ps.tile([128, 512], F32, tag="ps", name="prsh")
            ph0 = ps.tile([128, 512], F32, tag="ps", name="ph0")
            ph1 = ps.tile([128, 512], F32, tag="ps", name="ph1")
            acc04 = [
                ps.tile([128, 512], F32, tag="ps", name=f"acc0_{n}")
                for n in range(5)
            ]
            # both blocks' router logits share one bank at partition offsets
            # 0 and 32 (legal matmul tile positions). PSUM start=True zeroing
            # is bank-granular, so the bank is zeroed once by the DVE and
            # every router matmul accumulates with start=False.
            pr0 = prsh[0:N_EXP, :]
            pr1 = prsh[32:32 + N_EXP, :]
            nc.vector.memset(prsh, 0.0)

            def base_mms(acc, n, wtiles, w8tiles, close=False):
                """Full mixed-precision base K-loop for token tile n."""
                nsl = slice(n * 128, (n + 1) * 128)
                for k in range(KB):
                    mm(acc, xts[k][:, nsl], wtiles[k], start=(k == 0), stop=False)
                for t in range(KF):
                    mm(acc, x8_sb[:, t, :, nsl], w8tiles[t],
                       start=False, stop=False, perf_mode=DR)

            wb0_tiles = [wb0[:, k, :] for k in range(KB)]
            w80_tiles = [w80[:, t, :, :] for t in range(KF)]

            # ---- phase A: fused phase-1 + o0(n0..3) K-loop. o0's fp8
            # DoubleRow mms run in the first k-slots (their small operand
            # tensors arrive before the bf16 x stream ramps); o0's bf16 mms
            # trail phase-1 by 8 k-tiles in the FIFO so a late W-cache chunk
            # can never head-of-line-block the router/A stream. ----
            LAG = 8
            for ks in range(K_TILES + LAG):
                if ks < K_TILES:
                    k = ks
                    sp = (k == K_TILES - 1)
                    mm(pr0, wrt_sb[:, k, :], xts[k][:, 0:512],
                       start=False, stop=sp, skip_group_check=True)
                    if k < KB:
                        mm(ph0, at_sb[:, k, :], xts[k][:, 0:512],
                           start=(k == 0), stop=False)
                    elif (k - KB) % 2 == 0:
                        t = (k - KB) // 2
                        mm(ph0, at8_sb[:, t, :, :], x8_sb[:, t, :, 0:512],
                           start=False, stop=(t == KF - 1), perf_mode=DR)
                    mm(pr1, wrt_sb[:, k, :], xts[k][:, 512:1024],
                       start=False, stop=sp, skip_group_check=True)
                    if k < KB:
                        mm(ph1, at_sb[:, k, :], xts[k][:, 512:1024],
                           start=(k == 0), stop=False)
                    elif (k - KB) % 2 == 0:
                        t = (k - KB) // 2
                        mm(ph1, at8_sb[:, t, :, :], x8_sb[:, t, :, 512:1024],
                           start=False, stop=(t == KF - 1), perf_mode=DR)
                if ks >= LAG:
                    kb = ks - LAG
                    if kb < KB:
                        for n in range(5):
                            mm(acc04[n], xts[kb][:, n * 128:(n + 1) * 128],
                               wb0_tiles[kb], start=(kb == 0), stop=False)
                    elif (kb - KB) % 2 == 0:
                        t = (kb - KB) // 2
                        for n in range(5):
                            mm(acc04[n], x8_sb[:, t, :, n * 128:(n + 1) * 128],
                               w80_tiles[t], start=False, stop=False,
                               perf_mode=DR)

            # ---- phase B allocations ----
            acc5 = ps.tile([128, 512], F32, tag="ps", name="acc5")
            acc6 = ps.tile([128, 512], F32, tag="ps", name="acc6")
            scr0 = ps.tile([128, 512], F32, tag="ps", name="scr0")

            # drain phase-1 PSUM to SBUF (frees slots 0-2)
            lg0 = rbig.tile([N_EXP, BLK], F32, tag="lg", name="lg0")
            nc.vector.tensor_copy(out=lg0, in_=pr0)
            lg1 = rbig.tile([N_EXP, BLK], F32, tag="lg", name="lg1")
            nc.vector.tensor_copy(out=lg1, in_=pr1)
            # A operands are pre-scaled x256 (e4m3 range); undo it here.
            hs0 = rbig.tile([128, BLK], F32, tag="hs", name="hs0")
            nc.vector.tensor_scalar_mul(hs0, ph0, 1.0 / S)
            hs1 = rbig.tile([128, BLK], F32, tag="hs", name="hs1")
            nc.vector.tensor_scalar_mul(hs1, ph1, 1.0 / S)

            # forward transposes: logits chunks to token-major [128, 8]
            ltoks = []
            for i in range(8):
                b, c = i // 4, i % 4
                lg = lg0 if b == 0 else lg1
                scr = scr0
                nc.tensor.transpose(
                    out=scr[:, :N_EXP],
                    in_=lg[:, c * 128:(c + 1) * 128],
                    identity=ident_sb[:N_EXP, :N_EXP],
                )
                lt = rsm.tile([128, N_EXP], F32, tag="lt", name=f"lt_{i}", bufs=8)
                nc.vector.tensor_copy(out=lt, in_=scr[:, :N_EXP])
                ltoks.append(lt)

            # top-2 renormalized softmax weights, exact algebra:
            #   m1 = max_e l; t = l - m1; m2 = max_e (t | top1 -> -inf)
            #   combine_e = [t >= m2] * exp(t) * sigmoid(-m2)
            combs = []
            for i, lt in enumerate(ltoks):
                m1 = rsm.tile([128, 1], F32, tag="m1")
                nc.vector.tensor_reduce(
                    m1, lt, axis=mybir.AxisListType.X, op=mybir.AluOpType.max
                )
                t = rsm.tile([128, N_EXP], F32, tag="t")
                nc.vector.tensor_scalar(
                    out=t, in0=lt, scalar1=m1, scalar2=None,
                    op0=mybir.AluOpType.subtract,
                )
                eq = rsm.tile([128, N_EXP], F32, tag="eq")
                nc.vector.tensor_scalar(
                    out=eq, in0=t, scalar1=0.0, scalar2=None,
                    op0=mybir.AluOpType.is_ge,
                )
                msk = rsm.tile([128, N_EXP], F32, tag="msk")
                nc.vector.scalar_tensor_tensor(
                    out=msk, in0=eq, scalar=-1e30, in1=t,
                    op0=mybir.AluOpType.mult, op1=mybir.AluOpType.add,
                )
                m2 = rsm.tile([128, 1], F32, tag="m2")
                nc.vector.tensor_reduce(
                    m2, msk, axis=mybir.AxisListType.X, op=mybir.AluOpType.max
                )
                rec = rsm.tile([128, 1], F32, tag="rec")
                nc.scalar.activation(
                    rec, m2, mybir.ActivationFunctionType.Sigmoid, scale=-1.0
                )
                et = rsm.tile([128, N_EXP], F32, tag="et")
                nc.scalar.activation(et, t, mybir.ActivationFunctionType.Exp)
                ge = rsm.tile([128, N_EXP], F32, tag="ge")
                nc.vector.tensor_scalar(
                    out=ge, in0=t, scalar1=m2, scalar2=None,
                    op0=mybir.AluOpType.is_ge,
                )
                w = rsm.tile([128, N_EXP], F32, tag="w")
                nc.vector.tensor_tensor(
                    out=w, in0=et, in1=ge, op=mybir.AluOpType.mult
                )
                comb = rsm.tile([128, N_EXP], F32, tag="comb",
                                name=f"comb_{i}", bufs=8)
                nc.vector.tensor_scalar_mul(comb, w, rec)
                combs.append(comb)

            # token tile 5 of o0 runs while the DVE chews the routing chains
            base_mms(acc5, 5, wb0_tiles, w80_tiles)

            # combine block: transpose back + expand across ranks + fold
            def combine_block(b, hs):
                combt = rbig.tile([N_EXP, BLK], BF16, tag="ct", name=f"ct_{b}")
                for c in range(4):
                    nc.tensor.transpose(
                        out=scr0[:N_EXP, :128], in_=combs[b * 4 + c],
                        identity=ident_sb,
                    )
                    nc.vector.tensor_copy(
                        out=combt[:, c * 128:(c + 1) * 128], in_=scr0[:N_EXP, :128]
                    )
                # combine_expT[j, n] = combT[j//16, n] via emat.T @ combT
                mm(scr0, emat_sb, combt, start=True, stop=True)
                nc.vector.tensor_tensor(
                    out=hwt_sb[:, b * BLK:(b + 1) * BLK], in0=hs, in1=scr0,
                    op=mybir.AluOpType.mult,
                )

            combine_block(0, hs0)
            base_mms(acc6, 6, wb0_tiles, w80_tiles)
            combine_block(1, hs1)

            def drain(acc, n, osl, name, war_close=True):
                osb = out_pool.tile([128, 512], F32, tag="ob", name=name)
                nc.vector.scalar_tensor_tensor(
                    out=osb, in0=acc, scalar=1.0 / S, in1=bias_sb[:, osl],
                    op0=mybir.AluOpType.mult, op1=mybir.AluOpType.add,
                )
                nc.sync.dma_start(
                    out=out_ap[n * 128:(n + 1) * 128, osl], in_=osb
                )
                if war_close:
                    # WAR closer: makes the DVE (not the outbound DMA queue)
                    # the releaser of this staging slot.
                    nc.vector.memset(osb[0:1, 0:1], 0.0)

            # close + drain o0 n0..6
            o0sl = slice(0, 512)
            accs_o0 = acc04 + [acc5, acc6]
            for n in range(7):
                mm(accs_o0[n], hwt_sb[:, n * 128:(n + 1) * 128],
                   bft_sb[:, o0sl], start=False, stop=True)
            for n in range(7):
                drain(accs_o0[n], n, o0sl, f"ob_0_{n}")

            # ---- o-tiles 1..7: streamed W, base-first accumulators. W comes
            # in 8-k-tile chunk DMAs (3 triggers per o-tile) so trigger
            # execution never paces the K-loop; bufs=2 bounds how far the
            # stream can run ahead of consumption (no startup HBM theft). ----
            for o in range(1, O_TILES):
                osl = slice(o * 512, (o + 1) * 512)
                w8s = []
                for t in range(KF):
                    w8t = w8_pool.tile([128, 2, 512], F8, tag="w8",
                                       name=f"w8_{o}_{t}")
                    nc.sync.dma_start(out=w8t, in_=w8_d[t, :, :, osl])
                    w8s.append(w8t)
                accs = [
                    ps.tile([128, 512], F32, tag="ps", name=f"acc_{o}_{n}")
                    for n in range(N_TILES)
                ]
                for kc, k0 in enumerate(range(0, KB, 8)):
                    kw = min(8, KB - k0)
                    wtc = wt_pool.tile([128, 8, 512], BF16, tag="wt",
                                       name=f"wt_{o}_{kc}",
                                       padded_shape=[128, 8, 512])
                    nc.scalar.dma_start(
                        out=wtc[:, 0:kw, :],
                        in_=wtb_d[k0:k0 + kw, :, osl].transpose([1, 0, 2]),
                    )
                    for n in range(N_TILES):
                        for kk in range(kw):
                            k = k0 + kk
                            mm(accs[n], xts[k][:, n * 128:(n + 1) * 128],
                               wtc[:, kk, :], start=(k == 0), stop=False)
                for n in range(N_TILES):
                    for t in range(KF):
                        mm(accs[n], x8_sb[:, t, :, n * 128:(n + 1) * 128],
                           w8s[t], start=False, stop=False, perf_mode=DR)
                # close+drain interleaved: DVE starts draining acc n while
                # the PE closes n+1, so PSUM slots release in a cascade.
                for n in range(N_TILES):
                    mm(accs[n], hwt_sb[:, n * 128:(n + 1) * 128],
                       bft_sb[:, osl], start=False, stop=True)
                    drain(accs[n], n, osl, f"ob_{o}_{n}")

            # ---- deferred o0 token tile 7 (from the persistent cache) ----
            accd = ps.tile([128, 512], F32, tag="ps", name="accd_7")
            base_mms(accd, 7, wb0_tiles, w80_tiles)
            mm(accd, hwt_sb[:, 7 * 128:8 * 128],
               bft_sb[:, o0sl], start=False, stop=True)
            drain(accd, 7, o0sl, "ob_d_7", war_close=False)

    if finalize:
        nc.finalize()
    _CACHE[key] = nc
    return nc


def _prep_inputs(x, W_base, b_base, W_router, A, B):
    """Shard + lay out inputs for the 8 cores. Returns list of in_maps."""
    import ml_dtypes
    bf16 = ml_dtypes.bfloat16
    f8 = ml_dtypes.float8_e4m3
    x = np.asarray(x)
    W_base = np.asarray(W_base)
    b_base = np.asarray(b_base)
    W_router = np.asarray(W_router)
    A = np.asarray(A)
    B = np.asarray(B)
    x_flat = np.ascontiguousarray(x, dtype=np.float32).reshape(-1, D_IN)

    wT = W_base.T.astype(np.float32, copy=False) * S       # [4096, 4096]
    wtb = np.ascontiguousarray(
        wT[:KB * 128].reshape(KB, 128, D_OUT).astype(bf16)
    )
    w8 = np.ascontiguousarray(
        np.clip(wT[KB * 128:], -240.0, 240.0)
        .reshape(KF, 2, 128, D_OUT)
        .transpose(0, 2, 1, 3)
        .astype(f8)
    )
    atm = (
        A.astype(np.float32, copy=False)
        .transpose(2, 0, 1)
        .reshape(D_IN, J)
    ) * S
    at = np.ascontiguousarray(
        atm[:KB * 128].reshape(KB, 128, J).transpose(1, 0, 2).astype(bf16)
    )
    at8 = np.ascontiguousarray(
        np.clip(atm[KB * 128:], -240.0, 240.0)
        .reshape(KF, 2, 128, J)
        .transpose(0, 2, 1, 3)
        .astype(f8)
    )
    wrt = (
        W_router.T.astype(np.float32, copy=False)
        .reshape(K_TILES, 128, N_EXP)
        .transpose(1, 0, 2)
    )
    bft = np.ascontiguousarray(
        (SCALING * S * B.astype(np.float32, copy=False).transpose(0, 2, 1)
         .reshape(J, D_OUT)).astype(bf16)
    )
    bvec = np.ascontiguousarray(b_base, dtype=np.float32)
    # packed bf16 residents: [:, :256] wrt, [:8, 256:384] emat
    pkb = np.zeros((128, 384), dtype=bf16)
    pkb[:, 0:256] = wrt.reshape(128, K_TILES * N_EXP).astype(bf16)
    pkb[0:N_EXP, 256:384] = np.repeat(
        np.eye(N_EXP, dtype=np.float32), R, axis=1
    ).astype(bf16)
    ident = np.eye(128, dtype=np.float32)

    in_maps = []
    for c in range(N_CORES):
        shard = x_flat[c * TOK:(c + 1) * TOK]              # [1024, 4096]
        shT = np.ascontiguousarray(shard.T)                # [4096, 1024]
        xt = shT.astype(bf16).reshape(K_TILES, 128, TOK)
        x8 = np.ascontiguousarray(
            np.clip(shT[KB * 128:], -240.0, 240.0)
            .reshape(KF, 2, 128, TOK)
            .transpose(0, 2, 1, 3)
            .astype(f8)
        )
        in_maps.append({
            "xt": xt, "x8": x8, "wtb": wtb, "w8": w8, "at": at, "at8": at8,
            "bft": bft, "bvec": bvec, "pkb": pkb, "ident": ident,
        })
    return in_maps


def _run(in_maps, trace=False, **kw):
    nc = _build_program()
    return run_bass_kernel_spmd(
        nc, in_maps, core_ids=list(range(N_CORES)), trace=trace, **kw
    )


def kernel(x, W_base, b_base, W_router, A, B):
    orig_shape = np.asarray(x).shape
    in_maps = _prep_inputs(x, W_base, b_base, W_router, A, B)
    res = _run(in_maps)
    shards = [res.results[c]["out"] for c in range(N_CORES)]
    out = np.concatenate(shards, axis=0)
    return out.reshape(*orig_shape[:-1], D_OUT).astype(np.float32, copy=False)


# revision 33
# speedup vs baseline: 1.1932x; 1.0132x over previous
"""MoE-LoRA layer (nn_MoELoRALayer) as a Bass/Tile kernel for 8 Trainium2 cores.

Computation (per token n):
    logits = x @ W_router.T                    # [N, 8]
    combine = renorm(top2(softmax(logits)))    # [N, 8]
    h       = x @ A_cat.T                      # [N, 128]   (8 experts x rank 16)
    hw      = h * combine_expanded             # [N, 128]
    out     = x @ W_base.T + b + 2.0 * hw @ B_cat.T

Sharding: data-parallel over tokens (1024 per core), weights replicated.

Structure (vs the all-bf16 baseline at 561us; this version ~454us):
  * Mixed-precision GEMMs: k-tiles 0-21 run bf16, k-tiles 22-31 run as 5
    fp8(e4m3) DoubleRow matmuls (2 k-tiles per instruction, 2x PE
    throughput) for both the base GEMM and the LoRA down-projection.
    W-side operands are pre-scaled x256 host-side so fp8 values sit in
    e4m3's normal range; the PSUM drain multiplies by 1/256 fused into the
    bias add (the A-path undoes it in the PSUM->SBUF copy). Host-side RNE
    quantization; measured end-to-end rel err 1.83e-2 (gate 2e-2).
  * Accumulators open with the base GEMM and close with the LoRA up-proj
    (stop=True), so no o-tile ever waits on the routing chain.
  * Phase 1 (router logits + LoRA down-proj, both 512-token blocks) is
    interleaved k-by-k with o-tile 0's base K-loop for token tiles 0-4;
    o0's mms trail phase-1 by 8 k-tiles in the PE FIFO so a late W-cache
    chunk cannot head-of-line-block the router stream. Both blocks' logits
    share ONE PSUM bank (partition offsets 0/32, DVE-zeroed once,
    start=False accumulation) leaving 5 banks for accumulators. After the
    k-loop, token tiles 5,6 run from a persistent o0 W cache while the
    routing math (DVE/ACT, sigmoid trick) completes under them; token
    tile 7 is deferred to the end. W streams in 1MB 8-k-tile chunk DMAs
    (3 triggers/o-tile) with bufs=2 so trigger rate never paces the loop
    and the stream cannot steal startup HBM bandwidth.

Host-side layout prep (part of sharding):
    xt   [32, 128, 1024]   = x_shard.T bf16, K-tile major
    x8   [5, 128, 2, 1024] = e4m3 x_shard.T for k 2816..4095, DoubleRow pairs
    wtb  [22, 128, 4096]   = (W_base.T * 256)[0:2816] bf16, K-tile major
    w8   [5, 128, 2, 4096] = e4m3 (W_base.T * 256)[2816:4096], DoubleRow pairs
    at   [128, 22, 128]    = (A_cat.T * 256)[0:2816] per K-tile (j = e*16+r)
    at8  [5, 128, 2, 128]  = e4m3 (A_cat.T * 256)[2816:4096], DoubleRow pairs
    bft  [128, 4096]       = 2.0 * 256 * B.transpose(0,2,1).reshape(128, 4096)
    pkb  [128, 384]        = packed W_router.T (per K-tile) + expansion matrix
    ident [128, 128]       = identity for PE transposes
"""

import numpy as np

import concourse.bacc as bacc
import concourse.bass as bass
import concourse.mybir as mybir
import concourse.tile as tile
from concourse.bass_utils import run_bass_kernel_spmd

N_CORES = 8
D_IN = 4096
D_OUT = 4096
N_EXP = 8
R = 16
J = N_EXP * R           # 128
SCALING = 2.0
S = 256.0               # PSUM pre-scale folded into all W-side operands
TOK = 1024              # tokens per core
K_TILES = D_IN // 128   # 32
KB = 22                 # bf16 k-tiles (0..21)
KF = 5                  # fp8 DoubleRow super-tiles (k-tiles 22..31, 2 per)
N_TILES = TOK // 128    # 8
O_TILES = D_OUT // 512  # 8
BLK = 512

F32 = mybir.dt.float32
BF16 = mybir.dt.bfloat16
F8 = mybir.dt.float8e4

_CACHE = {}


def _build_program(finalize=True):
    key = ("nc", finalize)
    if key in _CACHE:
        return _CACHE[key]

    nc = bacc.Bacc(trn_type="TRN2")

    xt_d = nc.dram_tensor("xt", [K_TILES, 128, TOK], BF16, kind="ExternalInput")
    x8_d = nc.dram_tensor("x8", [KF, 128, 2, TOK], F8, kind="ExternalInput")
    wtb_d = nc.dram_tensor("wtb", [KB, 128, D_OUT], BF16, kind="ExternalInput")
    w8_d = nc.dram_tensor("w8", [KF, 128, 2, D_OUT], F8, kind="ExternalInput")
    at_d = nc.dram_tensor("at", [128, KB, J], BF16, kind="ExternalInput")
    at8_d = nc.dram_tensor("at8", [KF, 128, 2, J], F8, kind="ExternalInput")
    bft_d = nc.dram_tensor("bft", [J, D_OUT], BF16, kind="ExternalInput")
    bvec_d = nc.dram_tensor("bvec", [D_OUT], F32, kind="ExternalInput")
    pkb_d = nc.dram_tensor("pkb", [128, 384], BF16, kind="ExternalInput")
    id_d = nc.dram_tensor("ident", [128, 128], F32, kind="ExternalInput")
    out_d = nc.dram_tensor("out", [TOK, D_OUT], F32, kind="ExternalOutput")

    out_ap = out_d[:]
    mm = nc.tensor.matmul
    DR = mybir.MatmulPerfMode.DoubleRow

    with tile.TileContext(nc) as tc:
        with (
            tc.tile_pool(name="xt_pool", bufs=8) as xt_pool,
            tc.tile_pool(name="res", bufs=1) as res,
            tc.tile_pool(name="wt_pool", bufs=2) as wt_pool,
            tc.tile_pool(name="w8_pool", bufs=6) as w8_pool,
            tc.tile_pool(name="out_pool", bufs=6) as out_pool,
            tc.tile_pool(name="rsm", bufs=2) as rsm,
            tc.tile_pool(name="rbig", bufs=2) as rbig,
            tc.tile_pool(name="ps", bufs=8, space="PSUM") as ps,
        ):
            # ---- resident loads: phase-1 operands first so the k-loop can
            # start as soon as the first xt chunk lands ----
            pkr = res.tile([128, 384], BF16)
            nc.sync.dma_start(out=pkr, in_=pkb_d[:])
            pkf = res.tile([128, 128], F32)
            nc.sync.dma_start(out=pkf, in_=id_d[:])
            at_sb = res.tile([128, KB, J], BF16)
            nc.sync.dma_start(out=at_sb[:, 0:16, :], in_=at_d[:, 0:16, :])
            at8_sb = res.tile([128, KF, 2, J], F8)
            for t in range(KF):
                nc.sync.dma_start(out=at8_sb[:, t, :, :], in_=at8_d[t])
            wrt_sb = pkr[:, 0:256].rearrange("p (k e) -> p k e", e=N_EXP)
            emat_sb = pkr[0:N_EXP, 256:384]
            ident_sb = pkf
            hwt_sb = res.tile([J, TOK], BF16)
            w80 = res.tile([128, KF, 2, 512], F8)
            x8_sb = res.tile([128, KF, 2, TOK], F8)

            # x stream: chunk 0 split per K-tile so the PE can start on k=0
            # without waiting a full 1MB chunk; later chunks 4 K-tiles wide,
            # alternating queues. o0's W cache DMAs interleave with the x
            # chunks on the scalar queue so neither stream starves the other.
            wb0 = res.tile([128, KB, 512], BF16)
            xts = []
            xt_chunks = []
            g0 = xt_pool.tile([128, 4, TOK], BF16, tag="xt", name="xt_0")
            for kk in range(4):
                nc.scalar.dma_start(
                    out=g0[:, kk, :], in_=xt_d[kk]
                )
                xts.append(g0[:, kk, :])
            xt_chunks.append(g0)
            for g in range(1, 8):
                t = xt_pool.tile([128, 4, TOK], BF16, tag="xt", name=f"xt_{g}")
                eng = nc.scalar if g % 2 == 0 else nc.sync
                eng.dma_start(out=t, in_=xt_d[g * 4:(g + 1) * 4].transpose([1, 0, 2]))
                if g % 2 == 0:
                    j = 6 * (g // 2 - 1)
                    nc.scalar.dma_start(
                        out=wb0[:, j:j + 6, :],
                        in_=wtb_d[j:j + 6, :, 0:512].transpose([1, 0, 2]),
                    )
                if g == 3:
                    nc.sync.dma_start(
                        out=at_sb[:, 16:KB, :], in_=at_d[:, 16:KB, :]
                    )
                for kk in range(4):
                    xts.append(t[:, kk, :])
                xt_chunks.append(t)
            nc.scalar.dma_start(
                out=wb0[:, 18:KB, :],
                in_=wtb_d[18:KB, :, 0:512].transpose([1, 0, 2]),
            )
            for t2 in range(KF):
                nc.sync.dma_start(out=x8_sb[:, t2, :, :], in_=x8_d[t2])
                nc.sync.dma_start(out=w80[:, t2, :, :], in_=w8_d[t2, :, :, 0:512])
            bft_sb = res.tile([J, D_OUT], BF16)
            nc.scalar.dma_start(out=bft_sb, in_=bft_d[:])
            bias_sb = res.tile([128, D_OUT], F32)
            nc.gpsimd.dma_start(
                out=bias_sb, in_=bvec_d[:].partition_broadcast(128)
            )
            # bias arrives on a SWDGE queue; observe it on the DVE clock once.
            btch = rsm.tile([1, 1], F32, tag="btch")
            nc.vector.tensor_copy(out=btch, in_=bias_sb[0:1, 0:1])

            # ---- PSUM slot plan (single tag, bufs=8; allocation order IS the
            # slot order):
            #   slots 0-3: pr0, ph0, pr1, ph1   (phase 1)
            #   slots 4-7: o0 accs n0..n3
            # then acc4->slot0, acc5->slot1, scr0->slot2, scr1->slot3, and all
            # later accs keep cycling 4,5,6,7,0,1,2,3.
            prsh = ps.tile([128, 512], F32, tag="ps", name="prsh")
            ph0 = ps.tile([128, 512], F32, tag="ps", name="ph0")
            ph1 = ps.tile([128, 512], F32, tag="ps", name="ph1")
            acc04 = [
                ps.tile([128, 512], F32, tag="ps", name=f"acc0_{n}")
                for n in range(5)
            ]
            # both blocks' router logits share one bank at partition offsets
            # 0 and 32 (legal matmul tile positions). PSUM start=True zeroing
            # is bank-granular, so the bank is zeroed once by the DVE and
            # every router matmul accumulates with start=False.
            pr0 = prsh[0:N_EXP, :]
            pr1 = prsh[32:32 + N_EXP, :]
            nc.vector.memset(prsh, 0.0)

            def base_mms(acc, n, wtiles, w8tiles, close=False):
                """Full mixed-precision base K-loop for token tile n."""
                nsl = slice(n * 128, (n + 1) * 128)
                for k in range(KB):
                    mm(acc, xts[k][:, nsl], wtiles[k], start=(k == 0), stop=False)
                for t in range(KF):
                    mm(acc, x8_sb[:, t, :, nsl], w8tiles[t],
                       start=False, stop=False, perf_mode=DR)

            wb0_tiles = [wb0[:, k, :] for k in range(KB)]
            w80_tiles = [w80[:, t, :, :] for t in range(KF)]

            # ---- phase A: fused phase-1 + o0(n0..3) K-loop. o0's fp8
            # DoubleRow mms run in the first k-slots (their small operand
            # tensors arrive before the bf16 x stream ramps); o0's bf16 mms
            # trail phase-1 by 8 k-tiles in the FIFO so a late W-cache chunk
            # can never head-of-line-block the router/A stream. ----
            LAG = 8
            for ks in range(K_TILES + LAG):
                if ks < K_TILES:
                    k = ks
                    sp = (k == K_TILES - 1)
                    mm(pr0, wrt_sb[:, k, :], xts[k][:, 0:512],
                       start=False, stop=sp, skip_group_check=True)
                    if k < KB:
                        mm(ph0, at_sb[:, k, :], xts[k][:, 0:512],
                           start=(k == 0), stop=False)
                    elif (k - KB) % 2 == 0:
                        t = (k - KB) // 2
                        mm(ph0, at8_sb[:, t, :, :], x8_sb[:, t, :, 0:512],
                           start=False, stop=(t == KF - 1), perf_mode=DR)
                    mm(pr1, wrt_sb[:, k, :], xts[k][:, 512:1024],
                       start=False, stop=sp, skip_group_check=True)
                    if k < KB:
                        mm(ph1, at_sb[:, k, :], xts[k][:, 512:1024],
                           start=(k == 0), stop=False)
                    elif (k - KB) % 2 == 0:
                        t = (k - KB) // 2
                        mm(ph1, at8_sb[:, t, :, :], x8_sb[:, t, :, 512:1024],
                           start=False, stop=(t == KF - 1), perf_mode=DR)
                if ks >= LAG:
                    kb = ks - LAG
                    if kb < KB:
                        for n in range(5):
                            mm(acc04[n], xts[kb][:, n * 128:(n + 1) * 128],
                               wb0_tiles[kb], start=(kb == 0), stop=False)
                    elif (kb - KB) % 2 == 0:
                        t = (kb - KB) // 2
                        for n in range(5):
                            mm(acc04[n], x8_sb[:, t, :, n * 128:(n + 1) * 128],
                               w80_tiles[t], start=False, stop=False,
                               perf_mode=DR)

            # ---- phase B allocations ----
            acc5 = ps.tile([128, 512], F32, tag="ps", name="acc5")
            acc6 = ps.tile([128, 512], F32, tag="ps", name="acc6")
            scr0 = ps.tile([128, 512], F32, tag="ps", name="scr0")

            # drain phase-1 PSUM to SBUF (frees slots 0-2)
            lg0 = rbig.tile([N_EXP, BLK], F32, tag="lg", name="lg0")
            nc.vector.tensor_copy(out=lg0, in_=pr0)
            lg1 = rbig.tile([N_EXP, BLK], F32, tag="lg", name="lg1")
            nc.vector.tensor_copy(out=lg1, in_=pr1)
            # A operands are pre-scaled x256 (e4m3 range); undo it here.
            hs0 = rbig.tile([128, BLK], F32, tag="hs", name="hs0")
            nc.vector.tensor_scalar_mul(hs0, ph0, 1.0 / S)
            hs1 = rbig.tile([128, BLK], F32, tag="hs", name="hs1")
            nc.vector.tensor_scalar_mul(hs1, ph1, 1.0 / S)

            # forward transposes: logits chunks to token-major [128, 8]
            ltoks = []
            for i in range(8):
                b, c = i // 4, i % 4
                lg = lg0 if b == 0 else lg1
                scr = scr0
                nc.tensor.transpose(
                    out=scr[:, :N_EXP],
                    in_=lg[:, c * 128:(c + 1) * 128],
                    identity=ident_sb[:N_EXP, :N_EXP],
                )
                lt = rsm.tile([128, N_EXP], F32, tag="lt", name=f"lt_{i}", bufs=8)
                nc.vector.tensor_copy(out=lt, in_=scr[:, :N_EXP])
                ltoks.append(lt)

            # top-2 renormalized softmax weights, exact algebra:
            #   m1 = max_e l; t = l - m1; m2 = max_e (t | top1 -> -inf)
            #   combine_e = [t >= m2] * exp(t) * sigmoid(-m2)
            combs = []
            for i, lt in enumerate(ltoks):
                m1 = rsm.tile([128, 1], F32, tag="m1")
                nc.vector.tensor_reduce(
                    m1, lt, axis=mybir.AxisListType.X, op=mybir.AluOpType.max
                )
                t = rsm.tile([128, N_EXP], F32, tag="t")
                nc.vector.tensor_scalar(
                    out=t, in0=lt, scalar1=m1, scalar2=None,
                    op0=mybir.AluOpType.subtract,
                )
                eq = rsm.tile([128, N_EXP], F32, tag="eq")
                nc.vector.tensor_scalar(
                    out=eq, in0=t, scalar1=0.0, scalar2=None,
                    op0=mybir.AluOpType.is_ge,
                )
                msk = rsm.tile([128, N_EXP], F32, tag="msk")
                nc.vector.scalar_tensor_tensor(
                    out=msk, in0=eq, scalar=-1e30, in1=t,
                    op0=mybir.AluOpType.mult, op1=mybir.AluOpType.add,
                )
                m2 = rsm.tile([128, 1], F32, tag="m2")
                nc.vector.tensor_reduce(
                    m2, msk, axis=mybir.AxisListType.X, op=mybir.AluOpType.max
                )
                rec = rsm.tile([128, 1], F32, tag="rec")
                nc.scalar.activation(
                    rec, m2, mybir.ActivationFunctionType.Sigmoid, scale=-1.0
                )
                et = rsm.tile([128, N_EXP], F32, tag="et")
                nc.scalar.activation(et, t, mybir.ActivationFunctionType.Exp)
                ge = rsm.tile([128, N_EXP], F32, tag="ge")
                nc.vector.tensor_scalar(
                    out=ge, in0=t, scalar1=m2, scalar2=None,
                    op0=mybir.AluOpType.is_ge,
                )
                w = rsm.tile([128, N_EXP], F32, tag="w")
                nc.vector.tensor_tensor(
                    out=w, in0=et, in1=ge, op=mybir.AluOpType.mult
                )
                comb = rsm.tile([128, N_EXP], F32, tag="comb",
                                name=f"comb_{i}", bufs=8)
                nc.vector.tensor_scalar_mul(comb, w, rec)
                combs.append(comb)

            # token tile 5 of o0 runs while the DVE chews the routing chains
            base_mms(acc5, 5, wb0_tiles, w80_tiles)

            # combine block: transpose back + expand across ranks + fold
            def combine_block(b, hs):
                combt = rbig.tile([N_EXP, BLK], BF16, tag="ct", name=f"ct_{b}")
                for c in range(4):
                    nc.tensor.transpose(
                        out=scr0[:N_EXP, :128], in_=combs[b * 4 + c],
                        identity=ident_sb,
                    )
                    nc.vector.tensor_copy(
                        out=combt[:, c * 128:(c + 1) * 128], in_=scr0[:N_EXP, :128]
                    )
                # combine_expT[j, n] = combT[j//16, n] via emat.T @ combT
                mm(scr0, emat_sb, combt, start=True, stop=True)
                nc.vector.tensor_tensor(
                    out=hwt_sb[:, b * BLK:(b + 1) * BLK], in0=hs, in1=scr0,
                    op=mybir.AluOpType.mult,
                )

            combine_block(0, hs0)
            base_mms(acc6, 6, wb0_tiles, w80_tiles)
            combine_block(1, hs1)

            def drain(acc, n, osl, name, war_close=True):
                osb = out_pool.tile([128, 512], F32, tag="ob", name=name)
                nc.vector.scalar_tensor_tensor(
                    out=osb, in0=acc, scalar=1.0 / S, in1=bias_sb[:, osl],
                    op0=mybir.AluOpType.mult, op1=mybir.AluOpType.add,
                )
                nc.sync.dma_start(
                    out=out_ap[n * 128:(n + 1) * 128, osl], in_=osb
                )
                if war_close:
                    # WAR closer: makes the DVE (not the outbound DMA queue)
                    # the releaser of this staging slot.
                    nc.vector.memset(osb[0:1, 0:1], 0.0)

            # close + drain o0 n0..6
            o0sl = slice(0, 512)
            accs_o0 = acc04 + [acc5, acc6]
            for n in range(7):
                mm(accs_o0[n], hwt_sb[:, n * 128:(n + 1) * 128],
                   bft_sb[:, o0sl], start=False, stop=True)
            for n in range(7):
                drain(accs_o0[n], n, o0sl, f"ob_0_{n}")

            # ---- o-tiles 1..7: streamed W, base-first accumulators. W comes
            # in 8-k-tile chunk DMAs (3 triggers per o-tile) so trigger
            # execution never paces the K-loop; bufs=2 bounds how far the
            # stream can run ahead of consumption (no startup HBM theft). ----
            for o in range(1, O_TILES):
                osl = slice(o * 512, (o + 1) * 512)
                w8s = []
                for t in range(KF):
                    w8t = w8_pool.tile([128, 2, 512], F8, tag="w8",
                                       name=f"w8_{o}_{t}")
                    nc.sync.dma_start(out=w8t, in_=w8_d[t, :, :, osl])
                    w8s.append(w8t)
                accs = [
                    ps.tile([128, 512], F32, tag="ps", name=f"acc_{o}_{n}")
                    for n in range(N_TILES)
                ]
                for kc, k0 in enumerate(range(0, KB, 8)):
                    kw = min(8, KB - k0)
                    wtc = wt_pool.tile([128, 8, 512], BF16, tag="wt",
                                       name=f"wt_{o}_{kc}",
                                       padded_shape=[128, 8, 512])
                    nc.scalar.dma_start(
                        out=wtc[:, 0:kw, :],
                        in_=wtb_d[k0:k0 + kw, :, osl].transpose([1, 0, 2]),
                    )
                    for n in range(N_TILES):
                        for kk in range(kw):
                            k = k0 + kk
                            mm(accs[n], xts[k][:, n * 128:(n + 1) * 128],
                               wtc[:, kk, :], start=(k == 0), stop=False)
                for n in range(N_TILES):
                    for t in range(KF):
                        mm(accs[n], x8_sb[:, t, :, n * 128:(n + 1) * 128],
                           w8s[t], start=False, stop=False, perf_mode=DR)
                # close+drain interleaved: DVE starts draining acc n while
                # the PE closes n+1, so PSUM slots release in a cascade.
                for n in range(N_TILES):
                    mm(accs[n], hwt_sb[:, n * 128:(n + 1) * 128],
                       bft_sb[:, osl], start=False, stop=True)
                    drain(accs[n], n, osl, f"ob_{o}_{n}")

            # ---- deferred o0 token tile 7 (from the persistent cache) ----
            accd = ps.tile([128, 512], F32, tag="ps", name="accd_7")
            base_mms(accd, 7, wb0_tiles, w80_tiles)
            mm(accd, hwt_sb[:, 7 * 128:8 * 128],
               bft_sb[:, o0sl], start=False, stop=True)
            drain(accd, 7, o0sl, "ob_d_7", war_close=False)

    if finalize:
        nc.finalize()
    _CACHE[key] = nc
    return nc


def _prep_inputs(x, W_base, b_base, W_router, A, B):
    """Shard + lay out inputs for the 8 cores. Returns list of in_maps."""
    import ml_dtypes
    bf16 = ml_dtypes.bfloat16
    f8 = ml_dtypes.float8_e4m3
    x = np.asarray(x)
    W_base = np.asarray(W_base)
    b_base = np.asarray(b_base)
    W_router = np.asarray(W_router)
    A = np.asarray(A)
    B = np.asarray(B)
    x_flat = np.ascontiguousarray(x, dtype=np.float32).reshape(-1, D_IN)

    wT = W_base.T.astype(np.float32, copy=False) * S       # [4096, 4096]
    wtb = np.ascontiguousarray(
        wT[:KB * 128].reshape(KB, 128, D_OUT).astype(bf16)
    )
    w8 = np.ascontiguousarray(
        np.clip(wT[KB * 128:], -240.0, 240.0)
        .reshape(KF, 2, 128, D_OUT)
        .transpose(0, 2, 1, 3)
        .astype(f8)
    )
    atm = (
        A.astype(np.float32, copy=False)
        .transpose(2, 0, 1)
        .reshape(D_IN, J)
    ) * S
    at = np.ascontiguousarray(
        atm[:KB * 128].reshape(KB, 128, J).transpose(1, 0, 2).astype(bf16)
    )
    at8 = np.ascontiguousarray(
        np.clip(atm[KB * 128:], -240.0, 240.0)
        .reshape(KF, 2, 128, J)
        .transpose(0, 2, 1, 3)
        .astype(f8)
    )
    wrt = (
        W_router.T.astype(np.float32, copy=False)
        .reshape(K_TILES, 128, N_EXP)
        .transpose(1, 0, 2)
    )
    bft = np.ascontiguousarray(
        (SCALING * S * B.astype(np.float32, copy=False).transpose(0, 2, 1)
         .reshape(J, D_OUT)).astype(bf16)
    )
    bvec = np.ascontiguousarray(b_base, dtype=np.float32)
    # packed bf16 residents: [:, :256] wrt, [:8, 256:384] emat
    pkb = np.zeros((128, 384), dtype=bf16)
    pkb[:, 0:256] = wrt.reshape(128, K_TILES * N_EXP).astype(bf16)
    pkb[0:N_EXP, 256:384] = np.repeat(
        np.eye(N_EXP, dtype=np.float32), R, axis=1
    ).astype(bf16)
    ident = np.eye(128, dtype=np.float32)

    in_maps = []
    for c in range(N_CORES):
        shard = x_flat[c * TOK:(c + 1) * TOK]              # [1024, 4096]
        shT = np.ascontiguousarray(shard.T)                # [4096, 1024]
        xt = shT.astype(bf16).reshape(K_TILES, 128, TOK)
        x8 = np.ascontiguousarray(
            np.clip(shT[KB * 128:], -240.0, 240.0)
            .reshape(KF, 2, 128, TOK)
            .transpose(0, 2, 1, 3)
            .astype(f8)
        )
        in_maps.append({
            "xt": xt, "x8": x8, "wtb": wtb, "w8": w8, "at": at, "at8": at8,
            "bft": bft, "bvec": bvec, "pkb": pkb, "ident": ident,
        })
    return in_maps


def _run(in_maps, trace=False, **kw):
    nc = _build_program()
    return run_bass_kernel_spmd(
        nc, in_maps, core_ids=list(range(N_CORES)), trace=trace, **kw
    )


def kernel(x, W_base, b_base, W_router, A, B):
    orig_shape = np.asarray(x).shape
    in_maps = _prep_inputs(x, W_base, b_base, W_router, A, B)
    res = _run(in_maps)
    shards = [res.results[c]["out"] for c in range(N_CORES)]
    out = np.concatenate(shards, axis=0)
    return out.reshape(*orig_shape[:-1], D_OUT).astype(np.float32, copy=False)
